# revision 20
# baseline (speedup 1.0000x reference)
"""Masked-softmax attention (B=8, NQ=1024, S=2048, D=512) on 8 TRN2 NeuronCores.

Data-parallel: one batch element per core. The mask-after-softmax +
renormalize of the reference collapses algebraically:

    out[q] = sum_s exp(S[q,s]) * m[q,s] * V[s] / sum_s exp(S[q,s]) * m[q,s]

(the softmax Z and any constant score offset cancel), so one exp pass and a
single final per-row scale suffice. Per-core pipeline, entirely in
transposed layout so no on-chip transposes are needed:

    S^T[s-tile, q] = sum_d K^T[d, s-tile] . Q^T[d, q]   (PE, fp16)
    E^T            = exp(S^T - 100)                      (ACT, PSUM->SBUF bf16)
    P^T            = E^T * mask^T                        (DVE, uint8 mask)
    r_acc         += P^T  (per-partition partials)       (DVE, fp32r)
    O^T[d-tile]   += V-tile^T-contraction @ P^T          (PE accumulate, bf16)
    R              = ones^T @ r_acc  (partition-sum,     (PE, one matmul,
                     replicated to all 128 partitions)    fp32r)
    O^T            = O^T * (1/R)                         (DVE, fast reciprocal)

The constant -100 offset replaces the softmax max-subtraction: scores are
N(0, sqrt(512)) so exp(S-100) neither overflows nor all-underflows, and the
offset cancels exactly in the renormalization.

Schedule:
- All inputs are pre-swizzled on the host into the exact SBUF layout
  ([partition, ...] contiguous blocks), so every input DMA is a full-rate
  contiguous burst (>=1KB per descriptor; the natural kT layout would give
  256B bursts = half-rate) with a minimal descriptor count.
- The DMA engines run saturated (~300B/ns/core, chip-limited with 8 cores
  streaming) through the first ~25us, so input DMA issues go on ONE queue
  (sync HWDGE) in strict earliest-deadline-first order -- service order
  then equals need order, and no deferrable byte can delay an urgent one.
  V trails K/M by the matmul2 lag; qt1/qt2 go last. The exception is the
  first K tile on the scalar queue (ahead of its exp-table load), which
  overlaps the qt0 transfer.
- q is split into chunks (512, 384, 128) -- big first so the initial DMA
  stream keeps up with the PE (a 512-wide chunk consumes one 0.375MB
  kt/m/v tile-set per 1.7us, matching the supply rate), small last so the
  post-matmul drain (normalize muls + stores + epilogue semaphore waits)
  scales down 4x.
- Within a chunk matmul2 lags matmul1 by LAG s-tiles; the last chunk
  finishes its final 4 s-tiles d-major so each output d-tile's PSUM
  accumulation stops 4 matmuls apart and its normalize+store overlaps the
  next d-tile's matmuls.
- Chunk-boundary engine balancing: a finished chunk's 4 PSUM-read
  normalize muls (DVE-only: gpsimd cannot touch PSUM) are deferred into
  the next chunk's first steps so they do not stall the next chunk's mask
  pipeline; the last chunk's masks run on the otherwise-idle gpsimd and
  its row-sum R comes from per-tile PE ones-matmuls instead of DVE
  accumulation, leaving the DVE free for exactly those deferred muls.
"""

import numpy as np
import ml_dtypes

import concourse.mybir as mybir
import concourse.tile as tile
from concourse import bacc
from concourse.bass_utils import run_bass_kernel_spmd

B, NQ, S, D = 8, 1024, 2048, 512
NCORES = 8

P = 128              # partition tile
N_ST = S // P        # 16 s-tiles
N_DT = D // P        # 4 d-tiles

# (q0, width) per chunk; 512-max (PSUM bank / moving-free limit)
CHUNKS = [(0, 512), (512, 384), (896, 128)]
N_WARM = 42          # PE warmup matmuls (clock ramp + DMA lead-in cover)
N_TAIL = 4           # s-tiles finished d-major at the very end
LAG = 3
GROUPS = [(0, 1), (1, 1), (2, 1), (3, 1), (4, 2), (6, 2), (8, 2), (10, 2),
          (12, 2), (14, 2)]

F32 = mybir.dt.float32
F32R = mybir.dt.float32r
F16 = mybir.dt.float16
BF16 = mybir.dt.bfloat16
U8 = mybir.dt.uint8
EXP_OFFSET = -100.0


def build_nc():
    nc = bacc.Bacc("TRN2", target_bir_lowering=False, debug=False,
                   num_devices=NCORES)
    # Host-swizzled inputs: [128, ...] partition-major, fully contiguous per
    # DMA slice. qTp: per chunk [p][di][q'], kTp: per group [p][di][s'],
    # vp: per group [p][sl][d], mp: per group [p][sl][q].
    qTp = nc.declare_dram_parameter("qTp", [P, N_DT * NQ], F16, isOutput=False)
    kTp = nc.declare_dram_parameter("kTp", [P, N_DT * S], F16, isOutput=False)
    vp = nc.declare_dram_parameter("vp", [P, N_ST * D], BF16, isOutput=False)
    mp = nc.declare_dram_parameter("mp", [P, N_ST * NQ], U8, isOutput=False)
    oT = nc.declare_dram_parameter("oT", [D, NQ], F32, isOutput=True)

    with tile.TileContext(nc) as tc:
        with (
            tc.tile_pool(name="consts", bufs=1) as consts,
            tc.tile_pool(name="qt", bufs=1) as qt_pool,
            tc.tile_pool(name="kt", bufs=1) as kt_pool,
            tc.tile_pool(name="vp", bufs=1) as v_pool,
            tc.tile_pool(name="mp", bufs=1) as m_pool,
            tc.tile_pool(name="e", bufs=4) as e_pool,
            tc.tile_pool(name="p", bufs=8) as p_pool,
            tc.tile_pool(name="osb", bufs=4) as o_pool,
            tc.tile_pool(name="rec", bufs=2) as r_pool,
            tc.tile_pool(name="ps_s", bufs=4, space="PSUM") as ps_s,
            tc.tile_pool(name="ps_o", bufs=4, space="PSUM") as ps_o,
        ):
            ones_f32 = consts.tile([P, P], F32)
            nc.vector.memset(ones_f32[:, :], 1.0)
            ones_t = consts.tile([P, P], F32R)
            nc.vector.tensor_copy(ones_t[:, :], ones_f32[:, :])
            ones_bf = consts.tile([P, P], BF16)
            nc.vector.memset(ones_bf[:, :], 1.0)
            warm_t = consts.tile([P, P], BF16)
            nc.gpsimd.memset(warm_t[:, :], 0.0)
            bias_t = consts.tile([P, 1], F32)
            nc.vector.memset(bias_t[:, :], EXP_OFFSET)

            tile2grp = {}
            for gi, (gs, gn) in enumerate(GROUPS):
                for t in range(gn):
                    tile2grp[gs + t] = (gi, t)
            qt_sb = [qt_pool.tile([P, N_DT, w], F16, tag=f"qt{c}",
                                  name=f"qt{c}")
                     for c, (q0, w) in enumerate(CHUNKS)]
            kt_sb = [kt_pool.tile([P, N_DT, gn * P], F16, tag=f"kt{g}", name=f"kt{g}")
                     for g, (gs, gn) in enumerate(GROUPS)]
            v_sb = [v_pool.tile([P, gn, D], BF16, tag=f"v{g}", name=f"v{g}")
                    for g, (gs, gn) in enumerate(GROUPS)]
            m_sb = [m_pool.tile([P, gn, NQ], U8, tag=f"m{g}", name=f"m{g}")
                    for g, (gs, gn) in enumerate(GROUPS)]

            for w in range(N_WARM):
                wp = ps_s.tile([P, P], F32, name="warm_psum", tag="st")
                nc.tensor.matmul(wp[:, :], lhsT=warm_t[:, :], rhs=warm_t[:, :],
                                 start=True, stop=True)

            def dma_qt(eng, c, dlo, dhi):
                q0, w = CHUNKS[c]
                a = N_DT * q0 + dlo * w
                b = N_DT * q0 + dhi * w
                eng.dma_start(
                    out=qt_sb[c][:, dlo:dhi, :],
                    in_=qTp[:, a:b].rearrange("p (t q) -> p t q", t=dhi - dlo))

            def dma_kt(eng, g):
                gs, gn = GROUPS[g]
                a = N_DT * gs * P
                eng.dma_start(
                    out=kt_sb[g][:, :, :],
                    in_=kTp[:, a:a + N_DT * gn * P].rearrange(
                        "p (t s) -> p t s", t=N_DT))

            def dma_m(eng, g):
                gs, gn = GROUPS[g]
                a = gs * NQ
                eng.dma_start(
                    out=m_sb[g][:, :, :],
                    in_=mp[:, a:a + gn * NQ].rearrange("p (t q) -> p t q", t=gn))

            def dma_v(eng, g):
                gs, gn = GROUPS[g]
                a = gs * D
                eng.dma_start(
                    out=v_sb[g][:, :, :],
                    in_=vp[:, a:a + gn * D].rearrange("p (t d) -> p t d", t=gn))

            # Input DMA in strict earliest-deadline-first order. The DMA
            # engines run saturated through the first ~25us, so service
            # order ~= need order is what matters: any deferrable byte
            # issued early delays an urgent one behind it. v_g trails kt/m
            # by LAG tiles (matmul2 lag); qt1/qt2 are needed only at their
            # chunk starts and go last. Adjacent items alternate between
            # the sync HWDGE queue and the otherwise-idle gpsimd SWDGE
            # queue so round-robin descriptor service tracks the global
            # order; scalar gets only the first K tile ahead of its
            # exp-table load.
            dma_qt(nc.sync, 0, 0, N_DT)
            dma_kt(nc.scalar, 0)       # only scalar DMA: exp table follows
            dma_m(nc.sync, 0)
            dma_kt(nc.sync, 1)
            dma_m(nc.sync, 1)
            dma_kt(nc.sync, 2)
            dma_m(nc.sync, 2)
            dma_kt(nc.sync, 3)
            dma_m(nc.sync, 3)
            dma_v(nc.sync, 0)
            dma_kt(nc.sync, 4)
            dma_m(nc.sync, 4)
            dma_v(nc.sync, 1)
            dma_v(nc.sync, 2)
            dma_kt(nc.sync, 5)
            dma_m(nc.sync, 5)
            dma_v(nc.sync, 3)
            dma_v(nc.sync, 4)
            dma_kt(nc.sync, 6)
            dma_m(nc.sync, 6)
            dma_v(nc.sync, 5)
            dma_kt(nc.sync, 7)
            dma_m(nc.sync, 7)
            dma_v(nc.sync, 6)
            dma_kt(nc.sync, 8)
            dma_m(nc.sync, 8)
            dma_v(nc.sync, 7)
            dma_kt(nc.sync, 9)
            dma_m(nc.sync, 9)
            dma_v(nc.sync, 8)
            dma_v(nc.sync, 9)
            dma_qt(nc.sync, 1, 0, N_DT)
            dma_qt(nc.sync, 2, 0, N_DT)

            # Normalize+store of a finished chunk is deferred into the next
            # chunk's first steps: at a chunk boundary the DVE still owes
            # the 4 PSUM-read muls, and emitting them eagerly would stall
            # the next chunk's mask pipeline behind them.
            pending_finish = []
            for c, (q0, W) in enumerate(CHUNKS):
                last = (c == len(CHUNKS) - 1)
                ntail = N_TAIL if last else 0
                o_psum = [ps_o.tile([P, W], F32, name="o_psum")
                          for _ in range(N_DT)]
                r_acc = r_pool.tile([P, W], F32R, name="r_acc", tag="r_acc")
                p_tiles = {}
                # Software pipeline: matmul2 for s-tile (step-LAG) is emitted
                # after matmul1 for s-tile step, so the PE stream always has
                # independent work while exp/mask of the newest tile run.
                for step in range(N_ST + LAG):
                    if step < len(pending_finish):
                        pending_finish[step]()
                    if step < N_ST:
                        si = step
                        g, sl = tile2grp[si]
                        st = ps_s.tile([P, W], F32)
                        for di in range(N_DT):
                            nc.tensor.matmul(st[:, :],
                                             lhsT=kt_sb[g][:, di, sl * P:(sl + 1) * P],
                                             rhs=qt_sb[c][:, di, :],
                                             start=(di == 0), stop=(di == N_DT - 1))
                        e_t = e_pool.tile([P, W], BF16)
                        nc.scalar.activation(out=e_t[:, :], in_=st[:, :],
                                             func=mybir.ActivationFunctionType.Exp,
                                             bias=bias_t[:, 0:1], scale=1.0)
                        p_t = p_pool.tile([P, W], BF16)
                        # Last chunk: mask on the idle gpsimd engine — its
                        # narrow tiles run at DVE-op-overhead pace, and the
                        # DVE must simultaneously drain the previous chunk's
                        # deferred PSUM muls.
                        meng = nc.gpsimd if last else nc.vector
                        meng.tensor_mul(p_t[:, :], e_t[:, :],
                                        m_sb[g][:, sl, q0:q0 + W])
                        # Row-sum partial accumulation on DVE (f32r so a
                        # single PE ones-matmul can finish the reduction).
                        # The last (small) chunk instead sums P on the PE
                        # via per-tile ones-matmuls: at its width the DVE
                        # would otherwise pace the whole chunk.
                        if not last:
                            if si == 0:
                                nc.vector.tensor_copy(r_acc[:, :], p_t[:, :])
                            else:
                                nc.vector.tensor_add(r_acc[:, :], r_acc[:, :],
                                                     p_t[:, :])
                        p_tiles[si] = p_t
                        if si == N_ST - 1 and not last:
                            # Partition-sum of r_acc via one ones-matmul
                            # (result replicated across all 128 partitions),
                            # emitted right after the last partial add so the
                            # reciprocal is ready well before the output
                            # scaling needs it.
                            r_psum = ps_s.tile([P, W], F32, name="r_psum",
                                               tag="st")
                            nc.tensor.matmul(r_psum[:, :], lhsT=ones_t[:, :],
                                             rhs=r_acc[:, :],
                                             start=True, stop=True)
                            recip = r_pool.tile([P, W], F32)
                            nc.vector.reciprocal_approx_fast(recip[:, :],
                                                             r_psum[:, :])
                    if step >= LAG:
                        sj = step - LAG
                        if sj >= N_ST - ntail:
                            continue
                        gj, slj = tile2grp[sj]
                        p_r = p_tiles[sj][:, :]
                        for di in range(N_DT):
                            nc.tensor.matmul(o_psum[di][:, :],
                                             lhsT=v_sb[gj][:, slj, di * P:(di + 1) * P],
                                             rhs=p_r,
                                             start=(sj == 0),
                                             stop=(not last and sj == N_ST - 1))
                        if last:
                            if sj == 0:
                                r_psum = ps_s.tile([P, W], F32, name="r_psum",
                                                   tag="st")
                            nc.tensor.matmul(r_psum[:, :], lhsT=ones_bf[:, :],
                                             rhs=p_r, start=(sj == 0),
                                             stop=False)
                        if not last:
                            p_tiles.pop(sj)
                if not last:
                    def make_finish(di, o_ps, rec, q0_, W_):
                        def fin():
                            o_sb = o_pool.tile([P, W_], F32, name="o_sb")
                            nc.vector.tensor_mul(o_sb[:, :], o_ps[:, :],
                                                 rec[:, :])
                            nc.sync.dma_start(
                                out=oT[di * P:(di + 1) * P, q0_:q0_ + W_],
                                in_=o_sb[:, :])
                        return fin
                    pending_finish = [make_finish(di, o_psum[di], recip, q0, W)
                                      for di in range(N_DT)]
                else:
                    # d-major finish: complete each output d-tile's PSUM
                    # accumulation over the last N_TAIL s-tiles, then
                    # immediately normalize+store it while the next d-tile's
                    # matmuls run. The remaining ones-matmuls interleave so
                    # the reciprocal is ready by the time d0 completes.
                    # Output stores alternate HWDGE sequencers so the
                    # ~600ns issue costs overlap pairwise.
                    tail = list(range(N_ST - N_TAIL, N_ST))

                    def sweep(di):
                        for sj in tail:
                            gj, slj = tile2grp[sj]
                            nc.tensor.matmul(
                                o_psum[di][:, :],
                                lhsT=v_sb[gj][:, slj, di * P:(di + 1) * P],
                                rhs=p_tiles[sj][:, :],
                                start=False, stop=(sj == N_ST - 1))

                    def ones_mm(sj, stop):
                        nc.tensor.matmul(r_psum[:, :], lhsT=ones_bf[:, :],
                                         rhs=p_tiles[sj][:, :],
                                         start=False, stop=stop)

                    sweep(0)
                    ones_mm(tail[0], False)
                    ones_mm(tail[1], False)
                    sweep(1)
                    ones_mm(tail[2], False)
                    sweep(2)
                    ones_mm(tail[3], True)
                    recip = r_pool.tile([P, W], F32)
                    nc.vector.reciprocal_approx_fast(recip[:, :],
                                                     r_psum[:, :])
                    sweep(3)
                    for di in range(N_DT):
                        o_sb = o_pool.tile([P, W], F32)
                        nc.vector.tensor_mul(o_sb[:, :], o_psum[di][:, :],
                                             recip[:, :])
                        eng = nc.scalar if di % 2 == 1 else nc.sync
                        eng.dma_start(
                            out=oT[di * P:(di + 1) * P, q0:q0 + W],
                            in_=o_sb[:, :])
    nc.compile()
    return nc


_NC = None


def _get_nc():
    global _NC
    if _NC is None:
        _NC = build_nc()
    return _NC


def _swizzle(x, nrow):
    """[R*128, C] row-major -> [128, R*C]: blocks of 128 rows side by side,
    partition-major within each block."""
    r = x.shape[0] // nrow
    return np.ascontiguousarray(
        x.reshape(r, nrow, -1).transpose(1, 0, 2).reshape(nrow, -1))


def prep_in_maps(queries, keys, values, mask):
    queries = np.asarray(queries, dtype=np.float16)      # cast first: the
    keys = np.asarray(keys, dtype=np.float16)            # transpose copies
    mask = np.asarray(mask, dtype=np.uint8)              # then move 2-4x less
    values = np.asarray(values)
    in_maps = []
    for i in range(NCORES):
        # qTp: per chunk [p][di][q']  (chunk blocks at offset N_DT*q0)
        qprep = np.concatenate(
            [_swizzle(queries[i, q0:q0 + w, :].T, P) for q0, w in CHUNKS],
            axis=1)
        # kTp: per group [p][di][s']  (group blocks at offset N_DT*gs*P)
        kprep = np.concatenate(
            [_swizzle(keys[i, gs * P:(gs + gn) * P, :].T, P)
             for gs, gn in GROUPS], axis=1)
        # vp: [p][sl][d] — group blocks are contiguous in plain s-order
        vprep = _swizzle(values[i].astype(ml_dtypes.bfloat16), P)
        # mp: [p][sl][q]
        mprep = _swizzle(np.ascontiguousarray(mask[i].T), P)
        in_maps.append({"qTp": qprep, "kTp": kprep, "vp": vprep, "mp": mprep})
    return in_maps


def kernel(queries, keys, values, mask):
    nc = _get_nc()
    in_maps = prep_in_maps(queries, keys, values, mask)
    res = run_bass_kernel_spmd(nc, in_maps, core_ids=list(range(NCORES)))
    out = np.stack([res.results[i]["oT"].T for i in range(NCORES)])
    return np.ascontiguousarray(out, dtype=np.float32)


# revision 21
# speedup vs baseline: 1.0121x; 1.0121x over previous
"""Masked-softmax attention (B=8, NQ=1024, S=2048, D=512) on 8 TRN2 NeuronCores.

Data-parallel: one batch element per core. The mask-after-softmax +
renormalize of the reference collapses algebraically:

    out[q] = sum_s exp(S[q,s]) * m[q,s] * V[s] / sum_s exp(S[q,s]) * m[q,s]

(the softmax Z and any constant score offset cancel), so one exp pass and a
single final per-row scale suffice. Per-core pipeline, entirely in
transposed layout so no on-chip transposes are needed:

    S^T[s-tile, q] = sum_d K^T[d, s-tile] . Q^T[d, q]   (PE, fp16)
    E^T            = exp(S^T - 100)                      (ACT, PSUM->SBUF bf16)
    P^T            = E^T * mask^T                        (DVE, uint8 mask)
    r_acc         += P^T  (per-partition partials)       (DVE, fp32r)
    O^T[d-tile]   += V-tile^T-contraction @ P^T          (PE accumulate, bf16)
    R              = ones^T @ r_acc  (partition-sum,     (PE, one matmul,
                     replicated to all 128 partitions)    fp32r)
    O^T            = O^T * (1/R)                         (DVE, fast reciprocal)

The constant -100 offset replaces the softmax max-subtraction: scores are
N(0, sqrt(512)) so exp(S-100) neither overflows nor all-underflows, and the
offset cancels exactly in the renormalization.

Schedule:
- All inputs are pre-swizzled on the host into the exact SBUF layout
  ([partition, ...] contiguous blocks), so every input DMA is a full-rate
  contiguous burst (>=1KB per descriptor; the natural kT layout would give
  256B bursts = half-rate) with a minimal descriptor count.
- The DMA engines run saturated (~300B/ns/core, chip-limited with 8 cores
  streaming) through the first ~25us, so input DMA issues go on ONE queue
  (sync HWDGE) in strict earliest-deadline-first order -- service order
  then equals need order, and no deferrable byte can delay an urgent one.
  V trails K/M by the matmul2 lag; qt1/qt2 go last. The exception is the
  first K tile on the scalar queue (ahead of its exp-table load), which
  overlaps the qt0 transfer.
- q is split into chunks (512, 384, 128) -- big first so the initial DMA
  stream keeps up with the PE (a 512-wide chunk consumes one 0.375MB
  kt/m/v tile-set per 1.7us, matching the supply rate), small last so the
  post-matmul drain (normalize muls + stores + epilogue semaphore waits)
  scales down 4x.
- Within a chunk matmul2 lags matmul1 by LAG s-tiles; the last chunk
  finishes its final 4 s-tiles d-major so each output d-tile's PSUM
  accumulation stops 4 matmuls apart and its normalize+store overlaps the
  next d-tile's matmuls.
- Chunk-boundary engine balancing: a finished chunk's 4 PSUM-read
  normalize muls (DVE-only: gpsimd cannot touch PSUM) are deferred into
  the next chunk's first steps so they do not stall the next chunk's mask
  pipeline; the last chunk's masks run on the otherwise-idle gpsimd and
  its row-sum R comes from per-tile PE ones-matmuls instead of DVE
  accumulation, leaving the DVE free for exactly those deferred muls.
"""

import numpy as np
import ml_dtypes

import concourse.mybir as mybir
import concourse.tile as tile
from concourse import bacc
from concourse.bass_utils import run_bass_kernel_spmd

B, NQ, S, D = 8, 1024, 2048, 512
NCORES = 8

P = 128              # partition tile
N_ST = S // P        # 16 s-tiles
N_DT = D // P        # 4 d-tiles

# (q0, width) per chunk; 512-max (PSUM bank / moving-free limit)
CHUNKS = [(0, 512), (512, 384), (896, 128)]
N_WARM = 42          # PE warmup matmuls (clock ramp + DMA lead-in cover)
N_TAIL = 4           # s-tiles finished d-major at the very end
LAG = 3
GROUPS = [(0, 1), (1, 1), (2, 1), (3, 1), (4, 2), (6, 2), (8, 2), (10, 2),
          (12, 2), (14, 2)]

F32 = mybir.dt.float32
F32R = mybir.dt.float32r
F16 = mybir.dt.float16
BF16 = mybir.dt.bfloat16
U8 = mybir.dt.uint8
EXP_OFFSET = -100.0


def build_nc():
    nc = bacc.Bacc("TRN2", target_bir_lowering=False, debug=False,
                   num_devices=NCORES)
    # Host-swizzled inputs: [128, ...] partition-major, fully contiguous per
    # DMA slice. qTp: per chunk [p][di][q'], kTp: per group [p][di][s'],
    # vp: per group [p][sl][d], mp: per group [p][sl][q].
    qTp = nc.declare_dram_parameter("qTp", [P, N_DT * NQ], F16, isOutput=False)
    kTp = nc.declare_dram_parameter("kTp", [P, N_DT * S], F16, isOutput=False)
    vp = nc.declare_dram_parameter("vp", [P, N_ST * D], BF16, isOutput=False)
    mp = nc.declare_dram_parameter("mp", [P, N_ST * NQ], U8, isOutput=False)
    oT = nc.declare_dram_parameter("oT", [D, NQ], F32, isOutput=True)

    with tile.TileContext(nc) as tc:
        with (
            tc.tile_pool(name="consts", bufs=1) as consts,
            tc.tile_pool(name="qt", bufs=1) as qt_pool,
            tc.tile_pool(name="kt", bufs=1) as kt_pool,
            tc.tile_pool(name="vp", bufs=1) as v_pool,
            tc.tile_pool(name="mp", bufs=1) as m_pool,
            tc.tile_pool(name="e", bufs=4) as e_pool,
            tc.tile_pool(name="p", bufs=8) as p_pool,
            tc.tile_pool(name="osb", bufs=4) as o_pool,
            tc.tile_pool(name="rec", bufs=2) as r_pool,
            tc.tile_pool(name="ps_s", bufs=4, space="PSUM") as ps_s,
            tc.tile_pool(name="ps_o", bufs=4, space="PSUM") as ps_o,
        ):
            ones_f32 = consts.tile([P, P], F32)
            nc.vector.memset(ones_f32[:, :], 1.0)
            ones_t = consts.tile([P, P], F32R)
            nc.vector.tensor_copy(ones_t[:, :], ones_f32[:, :])
            ones_bf = consts.tile([P, P], BF16)
            nc.vector.memset(ones_bf[:, :], 1.0)
            warm_t = consts.tile([P, P], BF16)
            nc.gpsimd.memset(warm_t[:, :], 0.0)
            bias_t = consts.tile([P, 1], F32)
            nc.vector.memset(bias_t[:, :], EXP_OFFSET)

            tile2grp = {}
            for gi, (gs, gn) in enumerate(GROUPS):
                for t in range(gn):
                    tile2grp[gs + t] = (gi, t)
            qt_sb = [qt_pool.tile([P, N_DT, w], F16, tag=f"qt{c}",
                                  name=f"qt{c}")
                     for c, (q0, w) in enumerate(CHUNKS)]
            kt_sb = [kt_pool.tile([P, N_DT, gn * P], F16, tag=f"kt{g}", name=f"kt{g}")
                     for g, (gs, gn) in enumerate(GROUPS)]
            v_sb = [v_pool.tile([P, gn, D], BF16, tag=f"v{g}", name=f"v{g}")
                    for g, (gs, gn) in enumerate(GROUPS)]
            m_sb = [m_pool.tile([P, gn, NQ], U8, tag=f"m{g}", name=f"m{g}")
                    for g, (gs, gn) in enumerate(GROUPS)]

            for w in range(N_WARM):
                wp = ps_s.tile([P, P], F32, name="warm_psum", tag="st")
                nc.tensor.matmul(wp[:, :], lhsT=warm_t[:, :], rhs=warm_t[:, :],
                                 start=True, stop=True)

            def dma_qt(eng, c, dlo, dhi):
                q0, w = CHUNKS[c]
                a = N_DT * q0 + dlo * w
                b = N_DT * q0 + dhi * w
                eng.dma_start(
                    out=qt_sb[c][:, dlo:dhi, :],
                    in_=qTp[:, a:b].rearrange("p (t q) -> p t q", t=dhi - dlo))

            def dma_kt(eng, g):
                gs, gn = GROUPS[g]
                a = N_DT * gs * P
                eng.dma_start(
                    out=kt_sb[g][:, :, :],
                    in_=kTp[:, a:a + N_DT * gn * P].rearrange(
                        "p (t s) -> p t s", t=N_DT))

            def dma_m(eng, g):
                gs, gn = GROUPS[g]
                a = gs * NQ
                eng.dma_start(
                    out=m_sb[g][:, :, :],
                    in_=mp[:, a:a + gn * NQ].rearrange("p (t q) -> p t q", t=gn))

            def dma_v(eng, g):
                gs, gn = GROUPS[g]
                a = gs * D
                eng.dma_start(
                    out=v_sb[g][:, :, :],
                    in_=vp[:, a:a + gn * D].rearrange("p (t d) -> p t d", t=gn))

            # Input DMA in strict earliest-deadline-first order. The DMA
            # engines run saturated through the first ~25us, so service
            # order ~= need order is what matters: any deferrable byte
            # issued early delays an urgent one behind it. v_g trails kt/m
            # by LAG tiles (matmul2 lag); qt1/qt2 are needed only at their
            # chunk starts and go last. Adjacent items alternate between
            # the sync HWDGE queue and the otherwise-idle gpsimd SWDGE
            # queue so round-robin descriptor service tracks the global
            # order; scalar gets only the first K tile ahead of its
            # exp-table load.
            dma_qt(nc.sync, 0, 0, N_DT)
            dma_kt(nc.scalar, 0)       # only scalar DMA: exp table follows
            dma_m(nc.sync, 0)
            dma_kt(nc.sync, 1)
            dma_m(nc.sync, 1)
            dma_kt(nc.sync, 2)
            dma_m(nc.sync, 2)
            dma_kt(nc.sync, 3)
            dma_m(nc.sync, 3)
            dma_v(nc.sync, 0)
            dma_kt(nc.sync, 4)
            dma_m(nc.sync, 4)
            dma_v(nc.sync, 1)
            dma_v(nc.sync, 2)
            dma_kt(nc.sync, 5)
            dma_m(nc.sync, 5)
            dma_v(nc.sync, 3)
            dma_v(nc.sync, 4)
            dma_kt(nc.sync, 6)
            dma_m(nc.sync, 6)
            dma_v(nc.sync, 5)
            dma_kt(nc.sync, 7)
            dma_m(nc.sync, 7)
            dma_v(nc.sync, 6)
            dma_kt(nc.sync, 8)
            dma_m(nc.sync, 8)
            dma_v(nc.sync, 7)
            dma_kt(nc.sync, 9)
            dma_m(nc.sync, 9)
            dma_v(nc.sync, 8)
            dma_v(nc.sync, 9)
            dma_qt(nc.sync, 1, 0, N_DT)
            dma_qt(nc.sync, 2, 0, N_DT)

            # Normalize+store of a finished chunk is deferred into the next
            # chunk's first steps: at a chunk boundary the DVE still owes
            # the 4 PSUM-read muls, and emitting them eagerly would stall
            # the next chunk's mask pipeline behind them.
            pending_finish = []
            for c, (q0, W) in enumerate(CHUNKS):
                last = (c == len(CHUNKS) - 1)
                ntail = N_TAIL if last else 0
                o_psum = [ps_o.tile([P, W], F32, name="o_psum")
                          for _ in range(N_DT)]
                r_acc = r_pool.tile([P, W], F32R, name="r_acc", tag="r_acc")
                p_tiles = {}
                # Software pipeline: matmul2 for s-tile (step-LAG) is emitted
                # after matmul1 for s-tile step, so the PE stream always has
                # independent work while exp/mask of the newest tile run.
                for step in range(N_ST + LAG):
                    if step < len(pending_finish):
                        pending_finish[step]()
                    if step < N_ST:
                        si = step
                        g, sl = tile2grp[si]
                        st = ps_s.tile([P, W], F32)
                        for di in range(N_DT):
                            nc.tensor.matmul(st[:, :],
                                             lhsT=kt_sb[g][:, di, sl * P:(sl + 1) * P],
                                             rhs=qt_sb[c][:, di, :],
                                             start=(di == 0), stop=(di == N_DT - 1))
                        e_t = e_pool.tile([P, W], BF16)
                        nc.scalar.activation(out=e_t[:, :], in_=st[:, :],
                                             func=mybir.ActivationFunctionType.Exp,
                                             bias=bias_t[:, 0:1], scale=1.0)
                        p_t = p_pool.tile([P, W], BF16)
                        # Last chunk: masks alternate gpsimd/DVE so neither
                        # engine paces the chunk (gpsimd alone ran at
                        # ~415ns/op and gated the PE); the DVE still has the
                        # previous chunk's deferred PSUM muls early on, so
                        # even tiles go to gpsimd first.
                        if last:
                            meng = nc.gpsimd if si % 2 == 0 else nc.vector
                        else:
                            meng = nc.vector
                        meng.tensor_mul(p_t[:, :], e_t[:, :],
                                        m_sb[g][:, sl, q0:q0 + W])
                        # Row-sum partial accumulation on DVE (f32r so a
                        # single PE ones-matmul can finish the reduction).
                        # The last (small) chunk instead sums P on the PE
                        # via per-tile ones-matmuls: at its width the DVE
                        # would otherwise pace the whole chunk.
                        if not last:
                            if si == 0:
                                nc.vector.tensor_copy(r_acc[:, :], p_t[:, :])
                            else:
                                nc.vector.tensor_add(r_acc[:, :], r_acc[:, :],
                                                     p_t[:, :])
                        p_tiles[si] = p_t
                        if si == N_ST - 1 and not last:
                            # Partition-sum of r_acc via one ones-matmul
                            # (result replicated across all 128 partitions),
                            # emitted right after the last partial add so the
                            # reciprocal is ready well before the output
                            # scaling needs it.
                            r_psum = ps_s.tile([P, W], F32, name="r_psum",
                                               tag="st")
                            nc.tensor.matmul(r_psum[:, :], lhsT=ones_t[:, :],
                                             rhs=r_acc[:, :],
                                             start=True, stop=True)
                            recip = r_pool.tile([P, W], F32)
                            nc.vector.reciprocal_approx_fast(recip[:, :],
                                                             r_psum[:, :])
                    if step >= LAG:
                        sj = step - LAG
                        if sj >= N_ST - ntail:
                            continue
                        gj, slj = tile2grp[sj]
                        p_r = p_tiles[sj][:, :]
                        for di in range(N_DT):
                            nc.tensor.matmul(o_psum[di][:, :],
                                             lhsT=v_sb[gj][:, slj, di * P:(di + 1) * P],
                                             rhs=p_r,
                                             start=(sj == 0),
                                             stop=(not last and sj == N_ST - 1))
                        if last:
                            if sj == 0:
                                r_psum = ps_s.tile([P, W], F32, name="r_psum",
                                                   tag="st")
                            nc.tensor.matmul(r_psum[:, :], lhsT=ones_bf[:, :],
                                             rhs=p_r, start=(sj == 0),
                                             stop=False)
                        if not last:
                            p_tiles.pop(sj)
                if not last:
                    def make_finish(di, o_ps, rec, q0_, W_):
                        def fin():
                            o_sb = o_pool.tile([P, W_], F32, name="o_sb")
                            nc.vector.tensor_mul(o_sb[:, :], o_ps[:, :],
                                                 rec[:, :])
                            nc.sync.dma_start(
                                out=oT[di * P:(di + 1) * P, q0_:q0_ + W_],
                                in_=o_sb[:, :])
                        return fin
                    pending_finish = [make_finish(di, o_psum[di], recip, q0, W)
                                      for di in range(N_DT)]
                else:
                    # d-major finish: complete each output d-tile's PSUM
                    # accumulation over the last N_TAIL s-tiles, then
                    # immediately normalize+store it while the next d-tile's
                    # matmuls run. The remaining ones-matmuls interleave so
                    # the reciprocal is ready by the time d0 completes.
                    # Output stores alternate HWDGE sequencers so the
                    # ~600ns issue costs overlap pairwise.
                    tail = list(range(N_ST - N_TAIL, N_ST))

                    def sweep(di):
                        for sj in tail:
                            gj, slj = tile2grp[sj]
                            nc.tensor.matmul(
                                o_psum[di][:, :],
                                lhsT=v_sb[gj][:, slj, di * P:(di + 1) * P],
                                rhs=p_tiles[sj][:, :],
                                start=False, stop=(sj == N_ST - 1))

                    def ones_mm(sj, stop):
                        nc.tensor.matmul(r_psum[:, :], lhsT=ones_bf[:, :],
                                         rhs=p_tiles[sj][:, :],
                                         start=False, stop=stop)

                    sweep(0)
                    ones_mm(tail[0], False)
                    ones_mm(tail[1], False)
                    sweep(1)
                    ones_mm(tail[2], False)
                    sweep(2)
                    ones_mm(tail[3], True)
                    recip = r_pool.tile([P, W], F32)
                    nc.vector.reciprocal_approx_fast(recip[:, :],
                                                     r_psum[:, :])
                    sweep(3)
                    for di in range(N_DT):
                        o_sb = o_pool.tile([P, W], F32)
                        nc.vector.tensor_mul(o_sb[:, :], o_psum[di][:, :],
                                             recip[:, :])
                        eng = nc.scalar if di % 2 == 1 else nc.sync
                        eng.dma_start(
                            out=oT[di * P:(di + 1) * P, q0:q0 + W],
                            in_=o_sb[:, :])
    nc.compile()
    return nc


_NC = None


def _get_nc():
    global _NC
    if _NC is None:
        _NC = build_nc()
    return _NC


def _swizzle(x, nrow):
    """[R*128, C] row-major -> [128, R*C]: blocks of 128 rows side by side,
    partition-major within each block."""
    r = x.shape[0] // nrow
    return np.ascontiguousarray(
        x.reshape(r, nrow, -1).transpose(1, 0, 2).reshape(nrow, -1))


def prep_in_maps(queries, keys, values, mask):
    queries = np.asarray(queries, dtype=np.float16)      # cast first: the
    keys = np.asarray(keys, dtype=np.float16)            # transpose copies
    mask = np.asarray(mask, dtype=np.uint8)              # then move 2-4x less
    values = np.asarray(values)
    in_maps = []
    for i in range(NCORES):
        # qTp: per chunk [p][di][q']  (chunk blocks at offset N_DT*q0)
        qprep = np.concatenate(
            [_swizzle(queries[i, q0:q0 + w, :].T, P) for q0, w in CHUNKS],
            axis=1)
        # kTp: per group [p][di][s']  (group blocks at offset N_DT*gs*P)
        kprep = np.concatenate(
            [_swizzle(keys[i, gs * P:(gs + gn) * P, :].T, P)
             for gs, gn in GROUPS], axis=1)
        # vp: [p][sl][d] — group blocks are contiguous in plain s-order
        vprep = _swizzle(values[i].astype(ml_dtypes.bfloat16), P)
        # mp: [p][sl][q]
        mprep = _swizzle(np.ascontiguousarray(mask[i].T), P)
        in_maps.append({"qTp": qprep, "kTp": kprep, "vp": vprep, "mp": mprep})
    return in_maps


def kernel(queries, keys, values, mask):
    nc = _get_nc()
    in_maps = prep_in_maps(queries, keys, values, mask)
    res = run_bass_kernel_spmd(nc, in_maps, core_ids=list(range(NCORES)))
    out = np.stack([res.results[i]["oT"].T for i in range(NCORES)])
    return np.ascontiguousarray(out, dtype=np.float32)
